# revision 6
# baseline (speedup 1.0000x reference)
"""Masked scaled-dot-product attention (diagonal-excluded softmax) on 8 TRN2 cores.

Problem: q,k,v [32, 2048, 128] f32; diag_mask = ~eye (constant structure).
  attn = softmax((q @ k^T) / sqrt(128) with diag masked to -inf)  -> [32, L, L]
  out  = attn @ v                                                 -> [32, L, D]
Returns (out, attn), both float32, matching the reference.

Sharding: batch dim split across 8 cores (4 batches/core), no collectives.

Per-core dataflow (per batch):
  - load q,k natural; PE-transpose to qT,kT [d=128, L] (f32)
  - per 128-row block lb: S = qT[:,lb].T @ kT (fp32r matmuls, N=512)
    -> diag += -3e38 (DVE) -> exp (ACT, scale=1/T, accum_out=rowsum) -> E fp16
    -> attn_f32 = E * (1/rowsum)  (tensor_scalar) -> DMA out
    -> PE-transpose E (fp16, 1cyc/row) into attnT [m, l] via PSUM
  - AV: out'^T[dv, l] accumulation over m-blocks (lhsT = V fp16, rhs = attnT)
  - PE-transpose out'^T -> out[l, dv], scale by 1/rowsum during PSUM drain, DMA.
"""

import numpy as np

import concourse.bass as bass
import concourse.mybir as mybir
import concourse.tile as tile
from concourse import bacc
from concourse.masks import make_identity

TEMPERATURE = 11.313708498984761  # sqrt(128)
B, L_FULL, D = 32, 2048, 128
NCORES = 8
BPC = B // NCORES

F32 = mybir.dt.float32
F32R = mybir.dt.float32r
F16 = mybir.dt.float16


def build_nc(bpc=BPC, L=L_FULL):
    """Build the single-core Bass program for a [bpc, L, D] shard."""
    NL = L // 128          # number of 128-row blocks
    SC_W = min(1024, L)    # score psum tile width (2 banks max)
    NSC = L // SC_W        # score tiles per row-block
    NMM = SC_W // 512 if SC_W >= 512 else 1   # matmuls per score tile
    MMW = min(512, SC_W)   # matmul free dim
    TR_W = min(1024, NL * 128)  # transpose psum tile width (fp16 -> 1 bank)
    NTR = (NL * 128) // TR_W    # transpose groups per row-block
    TRB = TR_W // 128           # 128-col transposes per group
    NAV = (L + 511) // 512      # AV chunks

    nc = bacc.Bacc("TRN2", target_bir_lowering=False, debug=False,
                   num_devices=NCORES)

    q_d = nc.dram_tensor("q", [bpc, L, D], F32, kind="ExternalInput").ap()
    k_d = nc.dram_tensor("k", [bpc, L, D], F32, kind="ExternalInput").ap()
    v_d = nc.dram_tensor("v", [bpc, L, D], F32, kind="ExternalInput").ap()
    out_d = nc.dram_tensor("out", [bpc, L, D], F32, kind="ExternalOutput").ap()
    attn_d = nc.dram_tensor("attn", [bpc, L, L], F32, kind="ExternalOutput").ap()

    with tile.TileContext(nc) as tc:
        with (
            tc.tile_pool(name="const", bufs=1) as const_pool,
            tc.tile_pool(name="qkt", bufs=2) as qkt_pool,
            tc.tile_pool(name="stage", bufs=2) as stage_pool,
            tc.tile_pool(name="vp", bufs=2) as v_pool,
            tc.tile_pool(name="ep", bufs=2) as e_pool,
            tc.tile_pool(name="ap", bufs=2) as attn_pool,
            tc.tile_pool(name="att", bufs=1) as attnT_pool,
            tc.tile_pool(name="ot", bufs=1) as outT_pool,
            tc.tile_pool(name="os", bufs=2) as ostage_pool,
            tc.tile_pool(name="rs", bufs=2) as rs_pool,
            tc.tile_pool(name="scps", bufs=2, space="PSUM") as score_psum,
            tc.tile_pool(name="trps", bufs=2, space="PSUM") as tr_psum,
            tc.tile_pool(name="auxps", bufs=2, space="PSUM") as aux_psum,
        ):
            ident_f32 = const_pool.tile([128, 128], F32)
            make_identity(nc, ident_f32[:])
            ident_f16 = const_pool.tile([128, 128], F16)
            make_identity(nc, ident_f16[:])
            # -3e38 on the diagonal, 0 elsewhere (added to scores pre-exp)
            neg_ident = const_pool.tile([128, 128], F32)
            nc.gpsimd.memset(neg_ident[:], 0.0)
            nc.gpsimd.affine_select(
                out=neg_ident[:], in_=neg_ident[:],
                compare_op=mybir.AluOpType.not_equal,
                fill=-3e38, base=0, pattern=[[-1, 128]], channel_multiplier=1,
            )

            for ib in range(bpc):
                # ---- load + transpose q,k; cast-load v ----
                qn = stage_pool.tile([128, NL * 128], F32, tag="stage")
                kn = stage_pool.tile([128, NL * 128], F32, tag="stage")
                nc.sync.dma_start(
                    qn.rearrange("p (t d) -> p t d", t=NL),
                    q_d[ib].rearrange("(t p) d -> p t d", p=128))
                nc.sync.dma_start(
                    kn.rearrange("p (t d) -> p t d", t=NL),
                    k_d[ib].rearrange("(t p) d -> p t d", p=128))
                vt = v_pool.tile([128, NL * 128], F16)
                nc.gpsimd.dma_start(
                    vt.rearrange("p (t d) -> p t d", t=NL),
                    v_d[ib].rearrange("(t p) d -> p t d", p=128))

                qT = qkt_pool.tile([128, L], F32R, tag="qT")
                kT = qkt_pool.tile([128, L], F32R, tag="kT")
                for src, dst in ((qn, qT), (kn, kT)):
                    for g in range(NL // 4):
                        ps = aux_psum.tile([128, 512], F32, tag="aux")
                        for j in range(4):
                            t = g * 4 + j
                            nc.tensor.transpose(
                                ps[:, j * 128:(j + 1) * 128],
                                src[:, t * 128:(t + 1) * 128], ident_f32[:])
                        nc.vector.tensor_copy(
                            dst[:, g * 512:(g + 1) * 512], ps[:])

                attnT = attnT_pool.tile([128, NL * L], F16)
                attnT_v = attnT.rearrange("p (mb l) -> p mb l", mb=NL)
                recip_all = rs_pool.tile([128, NL], F32, tag="recip")

                # ---- score pass over row blocks ----
                for lb in range(NL):
                    E = e_pool.tile([128, L], F16)
                    rs_parts = rs_pool.tile([128, NSC], F32, tag="rsp")
                    for h in range(NSC):
                        sc = score_psum.tile([128, SC_W], F32)
                        for j in range(NMM):
                            nc.tensor.matmul(
                                sc[:, j * MMW:(j + 1) * MMW],
                                qT[:, lb * 128:(lb + 1) * 128],
                                kT[:, h * SC_W + j * MMW:
                                   h * SC_W + (j + 1) * MMW],
                                start=True, stop=True)
                        # mask the diagonal block if it lives in this tile
                        if h == (lb * 128) // SC_W:
                            c0 = lb * 128 - h * SC_W
                            nc.vector.tensor_tensor(
                                sc[:, c0:c0 + 128], sc[:, c0:c0 + 128],
                                neg_ident[:], mybir.AluOpType.add)
                        nc.scalar.activation(
                            E[:, h * SC_W:(h + 1) * SC_W], sc[:],
                            mybir.ActivationFunctionType.Exp,
                            scale=1.0 / TEMPERATURE,
                            accum_out=rs_parts[:, h:h + 1])
                    # rowsum -> reciprocal
                    rsum = rs_pool.tile([128, 1], F32, tag="rsum")
                    if NSC == 1:
                        nc.vector.reciprocal(recip_all[:, lb:lb + 1],
                                             rs_parts[:, 0:1])
                    else:
                        nc.vector.tensor_tensor(
                            rsum[:], rs_parts[:, 0:1], rs_parts[:, 1:2],
                            mybir.AluOpType.add)
                        for h in range(2, NSC):
                            nc.vector.tensor_tensor(
                                rsum[:], rsum[:], rs_parts[:, h:h + 1],
                                mybir.AluOpType.add)
                        nc.vector.reciprocal(recip_all[:, lb:lb + 1], rsum[:])

                    # attn (f32) = E * recip -> DMA out
                    at = attn_pool.tile([128, L], F32)
                    nc.vector.tensor_scalar_mul(at[:], E[:],
                                                recip_all[:, lb:lb + 1])
                    nc.sync.dma_start(attn_d[ib, lb * 128:(lb + 1) * 128, :],
                                      at[:])

                    # transpose E (fp16) into attnT[m, l]
                    for g in range(NTR):
                        trp = tr_psum.tile([128, TR_W], F16)
                        for j in range(TRB):
                            mb = g * TRB + j
                            nc.tensor.transpose(
                                trp[:, j * 128:(j + 1) * 128],
                                E[:, mb * 128:(mb + 1) * 128], ident_f16[:])
                        src_v = trp.rearrange("p (mb r) -> p mb r", mb=TRB)
                        dst_v = attnT_v[:, g * TRB:(g + 1) * TRB,
                                        lb * 128:(lb + 1) * 128]
                        if lb % 2 == 0:
                            nc.vector.tensor_copy(dst_v, src_v)
                        else:
                            nc.scalar.copy(dst_v, src_v)

                # ---- AV: out'^T [dv, l] accumulation over m blocks ----
                outT = outT_pool.tile([128, L], F32)
                for c in range(NAV):
                    po = aux_psum.tile([128, 512], F32, tag="aux")
                    w = min(512, L - c * 512)
                    for mb in range(NL):
                        nc.tensor.matmul(
                            po[:, :w],
                            vt[:, mb * 128:(mb + 1) * 128],
                            attnT_v[:, mb, c * 512:c * 512 + w],
                            start=(mb == 0), stop=(mb == NL - 1))
                    nc.vector.tensor_copy(outT[:, c * 512:c * 512 + w],
                                          po[:, :w])

                # ---- transpose out'^T -> out[l, dv], scale by recip ----
                ostage = ostage_pool.tile([128, NL * 128], F32)
                for lb in range(NL):
                    op = aux_psum.tile([128, 512], F32, tag="aux")
                    nc.tensor.transpose(op[:, :128],
                                        outT[:, lb * 128:(lb + 1) * 128],
                                        ident_f32[:])
                    nc.scalar.activation(
                        ostage[:, lb * 128:(lb + 1) * 128], op[:, :128],
                        mybir.ActivationFunctionType.Copy,
                        scale=recip_all[:, lb:lb + 1])
                nc.sync.dma_start(
                    out_d[ib].rearrange("(t p) d -> p t d", p=128),
                    ostage.rearrange("p (t d) -> p t d", t=NL))

    nc.compile()
    return nc


_nc_cache = {}


def _get_nc(bpc, L):
    key = (bpc, L)
    if key not in _nc_cache:
        _nc_cache[key] = build_nc(bpc, L)
    return _nc_cache[key]


def kernel(q, k, v, diag_mask=None, **kwargs):
    """Full-input entry point: shard batch over 8 cores, run, gather."""
    from concourse.bass_utils import run_bass_kernel_spmd

    q = np.ascontiguousarray(q, dtype=np.float32)
    k = np.ascontiguousarray(k, dtype=np.float32)
    v = np.ascontiguousarray(v, dtype=np.float32)
    bpc = q.shape[0] // NCORES
    nc = _get_nc(bpc, q.shape[1])

    in_maps = [
        {"q": q[i * bpc:(i + 1) * bpc],
         "k": k[i * bpc:(i + 1) * bpc],
         "v": v[i * bpc:(i + 1) * bpc]}
        for i in range(NCORES)
    ]
    res = run_bass_kernel_spmd(nc, in_maps, core_ids=list(range(NCORES)),
                               **kwargs)
    out = np.concatenate([res.results[i]["out"] for i in range(NCORES)], 0)
    attn = np.concatenate([res.results[i]["attn"] for i in range(NCORES)], 0)
    if kwargs.get("trace"):
        kernel.last_exec_time_ns = res.exec_time_ns
        kernel.last_results = res
    return out, attn


kernel.last_exec_time_ns = None
